# revision 1
# baseline (speedup 1.0000x reference)
"""Trainium2 Bass kernel: GPT2 block with ctx-pred sparse attention.

Sharding: tokens (blocks of 17) across 8 cores; window-head tokens
replicated. Each core runs the full block on its 272-token slice per
batch with zero collectives. All activations in transposed layout
[feature(partition), token(free)]; matmuls in bf16 with fp32 accum.
"""
import numpy as np
import ml_dtypes

import concourse.bass as bass
import concourse.mybir as mybir
import concourse.tile as tile
from concourse import bacc
from concourse.bass_utils import run_bass_kernel_spmd
from concourse.masks import make_identity

dt = mybir.dt
F32, BF16 = dt.float32, dt.bfloat16
AF = mybir.ActivationFunctionType
OP = mybir.AluOpType
BF = ml_dtypes.bfloat16

B = 2
S = 2176
HID = 768
NH = 12
DH = 64
WIN = 17
NCORE = 8
LTOK = 272           # local tokens per batch per core
NTOK = 400           # 272 local + 128 window heads
W2 = 2 * NTOK        # columns of xT: [b0 loc|b0 wh|b1 loc|b1 wh]
GROUPS = [(0, 119), (119, 119), (238, 34)]   # (q0, nq) block groups of 7,7,2
EPS = 1e-5
INNER = 3072

_CACHE = {}
LAST_RESULTS = None


def _body(tc, a):
    nc = tc.nc
    from contextlib import ExitStack
    ctx = ExitStack()
    P = 128

    sb = ctx.enter_context(tc.tile_pool(name="sb", bufs=1))
    sb2 = ctx.enter_context(tc.tile_pool(name="sb2", bufs=2))
    ps = ctx.enter_context(tc.tile_pool(name="ps", bufs=2, space="PSUM"))
    pat = ctx.enter_context(tc.tile_pool(name="pat", bufs=1, space="PSUM"))
    pst = ctx.enter_context(tc.tile_pool(name="pst", bufs=2, space="PSUM"))

    # ---- persistent sbuf ----
    xsb = sb.tile([P, 6 * W2], F32, tag="xsb")          # x^T
    xn = sb.tile([P, 6 * W2], BF16, tag="xn")           # ln1(x)^T bf16
    qt = sb.tile([P, 6 * 544], BF16, tag="qt")          # Q^T
    kt = sb.tile([P, 6 * W2], BF16, tag="kt")           # K^T
    vnat = sb.tile([P, 8 * HID], BF16, tag="vnat")      # V natural per (b,grp/wh)
    attnT = sb.tile([P, 6 * 544], BF16, tag="attnT")
    hid = sb.tile([P, 6 * 544], BF16, tag="hid")
    h2n = sb.tile([P, 6 * 544], BF16, tag="h2n")
    gelu = sb.tile([P, 24 * 544], BF16, tag="gelu")
    masksb = sb.tile([P, 768], F32, tag="masksb")
    wv = sb.tile([P, 6 * HID], BF16, tag="wv")
    wp = sb.tile([P, 6 * HID], BF16, tag="wp")
    ident = sb.tile([P, P], BF16, tag="ident")
    ones_col = sb.tile([P, 1], BF16, tag="ones_col")
    ones_row = sb.tile([1, P], BF16, tag="ones_row")
    ones_rowf = sb.tile([1, P], F32, tag="ones_rowf")
    bq_sb = sb.tile([P, 18], F32, tag="bq_sb")
    bap_sb = sb.tile([P, 6], F32, tag="bap_sb")
    bfc_sb = sb.tile([P, 24], F32, tag="bfc_sb")
    bmp_sb = sb.tile([P, 6], F32, tag="bmp_sb")
    bv_sb = sb.tile([1, HID], BF16, tag="bv_sb")
    eps_t = sb.tile([1, 1], F32, tag="eps_t")
    rows = sb.tile([1, 4 * W2], F32, tag="rows")        # mu|e2|var|rstd
    rows2 = sb.tile([1, 4 * 544], F32, tag="rows2")
    mu_b = sb.tile([P, W2], F32, tag="mu_b")
    rstd_b = sb.tile([P, W2], F32, tag="rstd_b")
    mu2_b = sb.tile([P, 544], F32, tag="mu2_b")
    rstd2_b = sb.tile([P, 544], F32, tag="rstd2_b")

    make_identity(nc, ident[:, :])
    nc.vector.memset(ones_col[:, :], 1.0)
    nc.vector.memset(ones_row[:, :], 1.0)
    nc.vector.memset(ones_rowf[:, :], 1.0)
    nc.vector.memset(eps_t[:, :], EPS)

    nc.sync.dma_start(xsb[:, :].rearrange("p (k n) -> p k n", k=6),
                      a["xT"].rearrange("(k p) n -> p k n", p=P))
    nc.sync.dma_start(masksb[:, :], a["mask"])
    nc.sync.dma_start(bq_sb[:, :], a["bqkv"].rearrange("(m p) -> p m", p=P))
    nc.sync.dma_start(bap_sb[:, :], a["bap"].rearrange("(m p) -> p m", p=P))
    nc.sync.dma_start(bfc_sb[:, :], a["bfc"].rearrange("(m p) -> p m", p=P))
    nc.sync.dma_start(bmp_sb[:, :], a["bmp"].rearrange("(m p) -> p m", p=P))
    nc.sync.dma_start(bv_sb[:, :], a["bv"].unsqueeze(0))
    nc.gpsimd.dma_start(wv[:, :].rearrange("p (k f) -> p k f", k=6),
                        a["wq"][:, 2 * HID:].rearrange("(k p) f -> p k f", p=P))
    nc.gpsimd.dma_start(wp[:, :].rearrange("p (k f) -> p k f", k=6),
                        a["wp"].rearrange("(k p) f -> p k f", p=P))

    def ln_stats(src, width, rows_t, mu_bt, rstd_bt, sq_from):
        # src: sbuf [128, 6*width] (f32 or bf16); per-column mean/rstd
        half = width // 2
        for h in range(2):
            s_ps = pst.tile([1, half], F32, tag="st")
            ss_ps = pst.tile([1, half], F32, tag="st")
            for k in range(6):
                c0 = width * k + half * h
                if sq_from == "cast":
                    cc = sb2.tile([P, half], BF16, tag="cchunk")
                    nc.scalar.activation(cc[:, :], src[:, c0:c0 + half], AF.Copy)
                else:
                    cc = None
                sqc = sb2.tile([P, half], BF16, tag="sqchunk")
                xin = cc[:, :] if cc is not None else src[:, c0:c0 + half]
                nc.scalar.activation(sqc[:, :], xin, AF.Square)
                nc.tensor.matmul(s_ps[:, :], ones_col[:, 0:1], xin,
                                 start=(k == 0), stop=(k == 5))
                nc.tensor.matmul(ss_ps[:, :], ones_col[:, 0:1], sqc[:, :],
                                 start=(k == 0), stop=(k == 5))
            mu = rows_t[0:1, half * h:half * (h + 1)]
            e2 = rows_t[0:1, width + half * h:width + half * (h + 1)]
            var = rows_t[0:1, 2 * width + half * h:2 * width + half * (h + 1)]
            rstd = rows_t[0:1, 3 * width + half * h:3 * width + half * (h + 1)]
            nc.vector.tensor_scalar_mul(mu, s_ps[:, :], 1.0 / HID)
            nc.vector.tensor_scalar_mul(e2, ss_ps[:, :], 1.0 / HID)
            nc.vector.tensor_tensor(var, mu, mu, OP.mult)
            nc.vector.tensor_tensor(var, e2, var, OP.subtract)
            nc.scalar.activation(var, var, AF.Ln, bias=eps_t[0:1, 0:1])
            nc.scalar.activation(rstd, var, AF.Exp, scale=-0.5)
        for h in range(2):
            for src_off, dstt in ((0, mu_bt), (3 * width, rstd_bt)):
                bc = ps.tile([P, NTOK], F32, tag="pp")
                nc.tensor.matmul(bc[:, 0:half], ones_rowf[0:1, 0:P],
                                 rows_t[0:1, src_off + half * h:src_off + half * (h + 1)],
                                 start=True, stop=True)
                nc.vector.tensor_copy(dstt[:, half * h:half * (h + 1)], bc[:, 0:half])

    def normalize(src, width, mu_bt, rstd_bt, dst):
        for k in range(6):
            c0 = width * k
            tmp = sb2.tile([P, width], F32, tag="lntmp")
            nc.vector.tensor_tensor(tmp[:, :], src[:, c0:c0 + width], mu_bt[:, :],
                                    OP.subtract)
            nc.vector.tensor_tensor(dst[:, c0:c0 + width], tmp[:, :], rstd_bt[:, :],
                                    OP.mult)

    # ---- LN1 ----
    ln_stats(xsb, W2, rows, mu_b, rstd_b, sq_from="cast")
    normalize(xsb, W2, mu_b, rstd_b, xn)

    # ---- QKV projections (Q^T, K^T) ----
    for m in range(12):
        wm = sb2.tile([P, 6 * P], BF16, tag="wtile")
        nc.gpsimd.dma_start(
            wm[:, :].rearrange("p (k f) -> p k f", k=6),
            a["wq"][:, P * m:P * (m + 1)].rearrange("(k p) f -> p k f", p=P))
        for b in range(2):
            n = LTOK if m < 6 else NTOK
            pp = ps.tile([P, NTOK], F32, tag="pp")
            for k in range(6):
                nc.tensor.matmul(pp[:, 0:n], wm[:, P * k:P * (k + 1)],
                                 xn[:, W2 * k + NTOK * b:W2 * k + NTOK * b + n],
                                 start=(k == 0), stop=(k == 5))
            if m < 6:
                dstap = qt[:, 544 * m + LTOK * b:544 * m + LTOK * b + n]
            else:
                dstap = kt[:, W2 * (m - 6) + NTOK * b:W2 * (m - 6) + NTOK * b + n]
            nc.scalar.activation(dstap, pp[:, 0:n], AF.Identity,
                                 bias=bq_sb[:, m:m + 1])

    # ---- V natural ----
    VT = GROUPS + [(LTOK, 128)]
    for b in range(2):
        for t, (t0, ntk) in enumerate(VT):
            for nh in range(2):
                vp = ps.tile([P, NTOK], F32, tag="pp")
                for k in range(6):
                    nc.tensor.matmul(
                        vp[0:ntk, 0:384],
                        xn[:, W2 * k + NTOK * b + t0:W2 * k + NTOK * b + t0 + ntk],
                        wv[:, HID * k + 384 * nh:HID * k + 384 * (nh + 1)],
                        start=(k == 0), stop=False)
                nc.tensor.matmul(vp[0:ntk, 0:384], ones_row[0:1, 0:ntk],
                                 bv_sb[0:1, 384 * nh:384 * (nh + 1)],
                                 start=False, stop=True)
                nc.vector.tensor_copy(
                    vnat[0:ntk, HID * (4 * b + t) + 384 * nh:
                         HID * (4 * b + t) + 384 * (nh + 1)], vp[0:ntk, 0:384])

    # ---- attention ----
    for b in range(2):
        for h in range(NH):
            ro = 64 * (h % 2)
            mm6 = h // 2
            for t, (q0, nq) in enumerate(GROUPS):
                kk = 128 + nq
                qts = qt[ro:ro + 64, 544 * mm6 + LTOK * b + q0:
                         544 * mm6 + LTOK * b + q0 + nq]
                sc = pat.tile([P, 256], F32, tag="sc")
                nc.tensor.matmul(sc[0:nq, 0:128], qts,
                                 kt[ro:ro + 64, W2 * mm6 + NTOK * b + LTOK:
                                    W2 * mm6 + NTOK * b + NTOK],
                                 start=True, stop=True)
                nc.tensor.matmul(sc[0:nq, 128:kk], qts,
                                 kt[ro:ro + 64, W2 * mm6 + NTOK * b + q0:
                                    W2 * mm6 + NTOK * b + q0 + nq],
                                 start=True, stop=True, skip_group_check=True)
                sm = sb2.tile([P, 256], F32, tag="sm")
                nc.vector.tensor_tensor(sm[0:nq, 0:kk], sc[0:nq, 0:kk],
                                        masksb[0:nq, 256 * t:256 * t + kk], OP.add)
                pe = sb2.tile([P, 256], BF16, tag="pe")
                lr = sb2.tile([P, 1], F32, tag="lr")
                nc.scalar.activation(pe[0:nq, 0:kk], sm[0:nq, 0:kk], AF.Exp,
                                     scale=0.125, accum_out=lr[0:nq, :])
                ri = sb2.tile([P, 1], F32, tag="ri")
                nc.vector.reciprocal(ri[0:nq, :], lr[0:nq, :])
                pn = sb2.tile([P, 256], BF16, tag="pn")
                nc.vector.tensor_scalar_mul(pn[0:nq, 0:kk], pe[0:nq, 0:kk],
                                            ri[0:nq, 0:1])
                pt1 = pat.tile([P, 119], BF16, tag="pt1")
                nc.tensor.transpose(pt1[0:128, 0:nq], pn[0:nq, 0:128],
                                    ident[0:nq, 0:nq])
                pt2 = pat.tile([P, 119], BF16, tag="pt2")
                nc.tensor.transpose(pt2[0:nq, 0:nq], pn[0:nq, 128:kk],
                                    ident[0:nq, 0:nq])
                pts1 = sb2.tile([P, 119], BF16, tag="pts1")
                nc.vector.tensor_copy(pts1[0:128, 0:nq], pt1[0:128, 0:nq])
                pts2 = sb2.tile([P, 119], BF16, tag="pts2")
                nc.vector.tensor_copy(pts2[0:nq, 0:nq], pt2[0:nq, 0:nq])
                av = pat.tile([64, 119], F32, tag="av")
                nc.tensor.matmul(av[:, 0:nq],
                                 vnat[0:128, HID * (4 * b + 3) + DH * h:
                                      HID * (4 * b + 3) + DH * (h + 1)],
                                 pts1[0:128, 0:nq], start=True, stop=False)
                nc.tensor.matmul(av[:, 0:nq],
                                 vnat[0:nq, HID * (4 * b + t) + DH * h:
                                      HID * (4 * b + t) + DH * (h + 1)],
                                 pts2[0:nq, 0:nq], start=False, stop=True,
                                 skip_group_check=True)
                nc.scalar.activation(
                    attnT[ro:ro + 64, 544 * mm6 + LTOK * b + q0:
                          544 * mm6 + LTOK * b + q0 + nq],
                    av[:, 0:nq], AF.Copy)

    # ---- attn_proj + residual -> hid (bf16) ----
    for m in range(6):
        for b in range(2):
            pp = ps.tile([P, NTOK], F32, tag="pp")
            for k in range(6):
                nc.tensor.matmul(pp[:, 0:LTOK], wp[:, HID * k + P * m:HID * k + P * (m + 1)],
                                 attnT[:, 544 * k + LTOK * b:544 * k + LTOK * b + LTOK],
                                 start=(k == 0), stop=(k == 5))
            nc.vector.scalar_tensor_tensor(
                hid[:, 544 * m + LTOK * b:544 * m + LTOK * b + LTOK],
                pp[:, 0:LTOK], bap_sb[:, m:m + 1],
                xsb[:, W2 * m + NTOK * b:W2 * m + NTOK * b + LTOK],
                op0=OP.add, op1=OP.add)

    # ---- LN2 ----
    ln_stats(hid, 544, rows2, mu2_b, rstd2_b, sq_from="direct")
    normalize(hid, 544, mu2_b, rstd2_b, h2n)

    # ---- fc + gelu ----
    for m in range(24):
        wm = sb2.tile([P, 6 * P], BF16, tag="wtile")
        nc.gpsimd.dma_start(
            wm[:, :].rearrange("p (k f) -> p k f", k=6),
            a["wf"][:, P * m:P * (m + 1)].rearrange("(k p) f -> p k f", p=P))
        for b in range(2):
            pp = ps.tile([P, NTOK], F32, tag="pp")
            for k in range(6):
                nc.tensor.matmul(pp[:, 0:LTOK], wm[:, P * k:P * (k + 1)],
                                 h2n[:, 544 * k + LTOK * b:544 * k + LTOK * b + LTOK],
                                 start=(k == 0), stop=(k == 5))
            nc.scalar.activation(gelu[:, 544 * m + LTOK * b:544 * m + LTOK * b + LTOK],
                                 pp[:, 0:LTOK], AF.Gelu_apprx_tanh,
                                 bias=bfc_sb[:, m:m + 1])

    # ---- mlp_proj + residual -> out ----
    for m in range(6):
        wmc = sb2.tile([P, 24 * P], BF16, tag="wmcol")
        nc.gpsimd.dma_start(
            wmc[:, :].rearrange("p (k f) -> p k f", k=24),
            a["wm"][:, P * m:P * (m + 1)].rearrange("(k p) f -> p k f", p=P))
        for b in range(2):
            pp = ps.tile([P, NTOK], F32, tag="pp")
            for k in range(24):
                nc.tensor.matmul(pp[:, 0:LTOK], wmc[:, P * k:P * (k + 1)],
                                 gelu[:, 544 * k + LTOK * b:544 * k + LTOK * b + LTOK],
                                 start=(k == 0), stop=(k == 23))
            ys = sb2.tile([P, LTOK], F32, tag="ys")
            nc.vector.scalar_tensor_tensor(
                ys[:, :], pp[:, 0:LTOK], bmp_sb[:, m:m + 1],
                hid[:, 544 * m + LTOK * b:544 * m + LTOK * b + LTOK],
                op0=OP.add, op1=OP.add)
            nc.sync.dma_start(a["yT"][P * m:P * (m + 1), LTOK * b:LTOK * (b + 1)],
                              ys[:, :])
    ctx.close()


def _build():
    nc = bacc.Bacc("TRN2", target_bir_lowering=False, debug=False)
    a = {}
    a["xT"] = nc.dram_tensor("xT", [HID, W2], F32, kind="ExternalInput").ap()
    a["mask"] = nc.dram_tensor("mask", [128, 768], F32, kind="ExternalInput").ap()
    a["wq"] = nc.dram_tensor("wq", [HID, 3 * HID], BF16, kind="ExternalInput").ap()
    a["wp"] = nc.dram_tensor("wp", [HID, HID], BF16, kind="ExternalInput").ap()
    a["wf"] = nc.dram_tensor("wf", [HID, INNER], BF16, kind="ExternalInput").ap()
    a["wm"] = nc.dram_tensor("wm", [INNER, HID], BF16, kind="ExternalInput").ap()
    a["bqkv"] = nc.dram_tensor("bqkv", [3 * HID], F32, kind="ExternalInput").ap()
    a["bv"] = nc.dram_tensor("bv", [HID], BF16, kind="ExternalInput").ap()
    a["bap"] = nc.dram_tensor("bap", [HID], F32, kind="ExternalInput").ap()
    a["bfc"] = nc.dram_tensor("bfc", [INNER], F32, kind="ExternalInput").ap()
    a["bmp"] = nc.dram_tensor("bmp", [HID], F32, kind="ExternalInput").ap()
    a["yT"] = nc.dram_tensor("yT", [HID, 2 * LTOK], F32, kind="ExternalOutput").ap()
    with tile.TileContext(nc) as tc:
        _body(tc, a)
    nc.compile()
    return nc


def _host_prep(inputs):
    x = np.ascontiguousarray(inputs["hidden_states"], np.float32)
    ln1_g = np.asarray(inputs["ln1_g"], np.float32)
    ln1_b = np.asarray(inputs["ln1_b"], np.float32)
    ln2_g = np.asarray(inputs["ln2_g"], np.float32)
    ln2_b = np.asarray(inputs["ln2_b"], np.float32)
    caw = np.asarray(inputs["c_attn_w"], np.float32)
    wq = (caw * ln1_g[:, None]).astype(BF)
    bqkv = (ln1_b @ caw + np.asarray(inputs["c_attn_b"], np.float32)).astype(np.float32)
    wp = np.asarray(inputs["attn_proj_w"], np.float32).astype(BF)
    fcw = np.asarray(inputs["fc_w"], np.float32)
    wf = (fcw * ln2_g[:, None]).astype(BF)
    bfc = (ln2_b @ fcw + np.asarray(inputs["fc_b"], np.float32)).astype(np.float32)
    wm = np.asarray(inputs["mlp_proj_w"], np.float32).astype(BF)
    shared = dict(
        wq=wq, wp=wp, wf=wf, wm=wm, bqkv=bqkv,
        bv=bqkv[2 * HID:].astype(BF),
        bap=np.asarray(inputs["attn_proj_b"], np.float32),
        bfc=bfc, bmp=np.asarray(inputs["mlp_proj_b"], np.float32))

    wh_idx = np.arange(128) * WIN
    in_maps = []
    for c in range(NCORE):
        t0 = LTOK * c
        cols = []
        for b in range(B):
            cols.append(np.concatenate([x[b, t0:t0 + LTOK], x[b, wh_idx]], 0))
        xT = np.ascontiguousarray(np.concatenate(cols, 0).T)
        mask = np.full((128, 768), -1e30, np.float32)
        for t, (q0, nq) in enumerate(GROUPS):
            for r in range(nq):
                blk = 16 * c + (q0 + r) // WIN
                mask[r, 256 * t:256 * t + blk] = 0.0
                base = 256 * t + 128 + (r // WIN) * WIN
                mask[r, base:base + r % WIN + 1] = 0.0
        in_maps.append(dict(shared, xT=xT, mask=mask))
    return in_maps


def kernel(**inputs):
    global LAST_RESULTS
    if "nc" not in _CACHE:
        _CACHE["nc"] = _build()
    nc = _CACHE["nc"]
    in_maps = _host_prep(inputs)
    res = run_bass_kernel_spmd(nc, in_maps, core_ids=list(range(NCORE)))
    LAST_RESULTS = res
    out = np.zeros((B, S, HID), np.float32)
    for c in range(NCORE):
        yT = res.results[c]["yT"]           # [768, 544]
        t0 = LTOK * c
        out[0, t0:t0 + LTOK] = yT[:, :LTOK].T
        out[1, t0:t0 + LTOK] = yT[:, LTOK:].T
    return out



# revision 13
# speedup vs baseline: 1.2387x; 1.2387x over previous
"""Trainium2 Bass kernel: GPT2 block with ctx-pred sparse attention.

Sharding: tokens (16 windows of 17) across 8 cores; 128 window-head
tokens replicated. Zero collectives. Activations transposed
[feature(part), token(free)], bf16 compute with fp32 psum accum.

Layout notes (per core):
- token columns per chunk: [b0 local 272 | b0 wh 128 | b1 local 272 | b1 wh 128]
- attention is k-major: scores [k, q]; softmax denom comes from a 65th
  "ones" feature column appended to each head's V; normalization =
  gpsimd partition-broadcast of 1/den + one DVE multiply.
- all weights are host-relaid so each DMA is one contiguous [128, X].
"""
import numpy as np
import ml_dtypes

import concourse.bass as bass
import concourse.mybir as mybir
import concourse.tile as tile
from concourse import bacc
from concourse.bass_utils import run_bass_kernel_spmd

dt = mybir.dt
F32, BF16 = dt.float32, dt.bfloat16
AF = mybir.ActivationFunctionType
OP = mybir.AluOpType
BF = ml_dtypes.bfloat16

B = 2
S = 2176
HID = 768
NH = 12
DH = 64
WIN = 17
NCORE = 8
LTOK = 272            # local tokens per batch per core
WH = 128              # replicated window-head tokens
NTOK = 400            # 272 local + 128 wh
COLS = 2 * NTOK       # 800 columns per feature chunk
GROUPS = [(0, 119), (119, 119), (238, 34)]
VBLK = GROUPS + [(272, 128)]     # (col0, ntk); col0=272 -> wh slot
VW = 65 * NH          # 780: v features + per-head ones column
EPS = 1e-5
INNER = 3072

_CACHE = {}
LAST_RESULTS = None

# bigA column carve (bf16 elems): xsb | qt | kt | maskw ; gelu aliases 0:13056
XSB0, QT0, KT0, MKW0, A_END = 0, 4800, 8064, 12864, 13136
GELU0 = 0


def _body(tc, a):
    nc = tc.nc
    from contextlib import ExitStack
    ctx = ExitStack()
    P = 128

    sb = ctx.enter_context(tc.tile_pool(name="sb", bufs=1))
    sq_p = ctx.enter_context(tc.tile_pool(name="sqp", bufs=2))
    tmp_p = ctx.enter_context(tc.tile_pool(name="tmpp", bufs=2))
    smw_p = ctx.enter_context(tc.tile_pool(name="smwp", bufs=2))
    smd_p = ctx.enter_context(tc.tile_pool(name="smdp", bufs=2))
    ew_p = ctx.enter_context(tc.tile_pool(name="ewp", bufs=2))
    ed_p = ctx.enter_context(tc.tile_pool(name="edp", bufs=2))
    rr_p = ctx.enter_context(tc.tile_pool(name="rrp", bufs=2))
    rbb_p = ctx.enter_context(tc.tile_pool(name="rbbp", bufs=2))
    ys_p = ctx.enter_context(tc.tile_pool(name="ysp", bufs=2))
    pp_p = ctx.enter_context(tc.tile_pool(name="ppp", bufs=2, space="PSUM"))
    stw_p = ctx.enter_context(tc.tile_pool(name="stwp", bufs=2, space="PSUM"))
    std_p = ctx.enter_context(tc.tile_pool(name="stdp", bufs=2, space="PSUM"))
    av_p = ctx.enter_context(tc.tile_pool(name="avp", bufs=2, space="PSUM"))

    # ---- persistent sbuf ----
    bigA = sb.tile([P, A_END], BF16, tag="bigA")       # xsb|qt|kt|maskw, later gelu
    xn = sb.tile([P, 6 * COLS], BF16, tag="xn")        # ln1(x)^T; h2n aliases [0:3264]
    vnat = sb.tile([P, 8 * VW], BF16, tag="vnat")      # V natural + ones cols
    attnT = sb.tile([P, 6 * 544], BF16, tag="attnT")
    hid = sb.tile([P, 6 * 544], BF16, tag="hid")
    bigW = sb.tile([P, 9216], BF16, tag="bigW")        # wqQ|wqK, later wmh2
    wvh = sb.tile([P, 6 * VW], BF16, tag="wvh")
    wph = sb.tile([P, 36 * P], BF16, tag="wph")
    wfh = sb.tile([P, 144 * P], BF16, tag="wfh")
    wmh1 = sb.tile([P, 72 * P], BF16, tag="wmh1")
    aux = sb.tile([P, 48], F32, tag="aux")             # bqk|bap2|bfc|bmp
    bv1 = sb.tile([1, VW], BF16, tag="bv1")            # ones-marker row for V
    maskd = sb.tile([P, LTOK], BF16, tag="maskd")
    ones_c = sb.tile([P, 1], BF16, tag="ones_c")
    ones_r = sb.tile([1, P], BF16, tag="ones_r")
    eps_t = sb.tile([1, 1], F32, tag="eps_t")
    frows = sb.tile([1, 3200], F32, tag="frows")       # mu|e2|mu2|var
    brows = sb.tile([1, 1600], BF16, tag="brows")      # mu_bf|rstd_bf
    mub = sb.tile([P, COLS], BF16, tag="mub")
    rsb = sb.tile([P, COLS], BF16, tag="rsb")

    xsb = bigA[:, XSB0:XSB0 + 4800]
    qt = bigA[:, QT0:QT0 + 3264]
    kt = bigA[:, KT0:KT0 + 4800]
    maskw = bigA[:, MKW0:MKW0 + LTOK]
    gelu = bigA[:, GELU0:GELU0 + 24 * 544]
    wqQ = bigW[:, 0:4608]
    wqK = bigW[:, 4608:9216]
    wmh2 = bigW[:, 0:9216]
    h2n = xn[:, 0:3264]

    nc.vector.memset(ones_c[:, :], 1.0)
    nc.vector.memset(ones_r[:, :], 1.0)
    nc.vector.memset(eps_t[:, :], EPS)

    # ---- input DMAs: sync=HWDGE ring A, scalar=HWDGE ring B, gpsimd=SWDGE
    nc.sync.dma_start(xsb, a["xT"])
    nc.sync.dma_start(wph[:, :], a["wph"])
    nc.sync.dma_start(wfh[:, :], a["wfh"])
    nc.sync.dma_start(wmh1[:, :], a["wmh1"])
    nc.scalar.dma_start(wqQ, a["wqQ"])
    nc.scalar.dma_start(wqK, a["wqK"])
    nc.scalar.dma_start(wvh[:, :], a["wvh"])
    nc.gpsimd.dma_start(aux[:, :], a["aux"])
    nc.gpsimd.dma_start(bv1[:, :], a["bv1"])
    nc.gpsimd.dma_start(maskd[:, :], a["maskd"])
    nc.gpsimd.dma_start(maskw, a["maskw"])

    def ln_stats(src, width, half):
        """Column mean/rstd of src [128, width] -> broadcast mub/rsb [:, :width]."""
        s0 = stw_p.tile([P, 512], F32, tag="stw")
        s1 = stw_p.tile([P, 512], F32, tag="stw")
        q0 = std_p.tile([P, 512], F32, tag="std")
        q1 = std_p.tile([P, 512], F32, tag="std")
        stats = [(s0, 0), (s1, half)]
        sqts = [(q0, 0), (q1, half)]
        for k in range(6):
            sq = sq_p.tile([P, COLS], BF16, tag="sq")
            chunk = src[:, width * k:width * (k + 1)]
            nc.vector.tensor_tensor(sq[:, 0:width], chunk, chunk, OP.mult)
            for (st, c0) in stats:
                nc.tensor.matmul(st[0:1, 0:half], ones_c[:, 0:1],
                                 src[:, width * k + c0:width * k + c0 + half],
                                 start=(k == 0), stop=(k == 5))
            for (st, c0) in sqts:
                nc.tensor.matmul(st[0:1, 0:half], ones_c[:, 0:1],
                                 sq[:, c0:c0 + half],
                                 start=(k == 0), stop=(k == 5))
        mur = frows[0:1, 0:width]
        e2r = frows[0:1, 800:800 + width]
        m2r = frows[0:1, 1600:1600 + width]
        varr = frows[0:1, 2400:2400 + width]
        for (st, c0) in stats:
            nc.vector.tensor_scalar_mul(mur[0:1, c0:c0 + half], st[0:1, 0:half],
                                        1.0 / HID)
        for (st, c0) in sqts:
            nc.vector.tensor_scalar_mul(e2r[0:1, c0:c0 + half], st[0:1, 0:half],
                                        1.0 / HID)
        nc.vector.tensor_tensor(m2r, mur, mur, OP.mult)
        nc.vector.tensor_tensor(varr, e2r, m2r, OP.subtract)
        murb = brows[0:1, 0:width]
        rstdb = brows[0:1, 800:800 + width]
        nc.vector.tensor_copy(murb, mur)
        lnv = frows[0:1, 1600:1600 + width]                   # reuse mu2 slot
        nc.scalar.activation(lnv, varr, AF.Ln, bias=eps_t[0:1, 0:1])
        nc.scalar.activation(rstdb, lnv, AF.Exp, scale=-0.5)
        nc.gpsimd.partition_broadcast(mub[:, 0:width], murb)
        nc.gpsimd.partition_broadcast(rsb[:, 0:width], rstdb)

    # ================= LN1 =================
    ln_stats(xsb, COLS, NTOK)
    for k in range(6):
        tmp = tmp_p.tile([P, COLS], BF16, tag="tmp")
        nc.vector.tensor_tensor(tmp[:, :], xsb[:, COLS * k:COLS * (k + 1)],
                                mub[:, :], OP.subtract)
        nc.vector.tensor_tensor(xn[:, COLS * k:COLS * (k + 1)], tmp[:, :],
                                rsb[:, :], OP.mult)

    # ================= QKV =================
    # Q: feature chunks m 0-5 -> qt, scale 1/8 pre-folded into bias
    for m in range(6):
        pps = [pp_p.tile([P, 512], F32, tag="pp", name=f"pp{i}") for i in range(2)]
        for k in range(6):
            w = wqQ[:, (m * 6 + k) * P:(m * 6 + k + 1) * P]
            for b in range(2):
                nc.tensor.matmul(pps[b][:, 0:LTOK], w,
                                 xn[:, COLS * k + NTOK * b:COLS * k + NTOK * b + LTOK],
                                 start=(k == 0), stop=(k == 5))
        for b in range(2):
            nc.scalar.activation(qt[:, 544 * m + LTOK * b:544 * m + LTOK * (b + 1)],
                                 pps[b][:, 0:LTOK], AF.Identity,
                                 bias=aux[:, m:m + 1], scale=0.125)
    # K: all 400 cols per batch -> kt
    for m in range(6):
        pps = [pp_p.tile([P, 512], F32, tag="pp", name=f"pp{i}") for i in range(2)]
        for k in range(6):
            w = wqK[:, (m * 6 + k) * P:(m * 6 + k + 1) * P]
            for b in range(2):
                nc.tensor.matmul(pps[b][:, 0:NTOK], w,
                                 xn[:, COLS * k + NTOK * b:COLS * k + NTOK * (b + 1)],
                                 start=(k == 0), stop=(k == 5))
        for b in range(2):
            nc.scalar.activation(kt[:, COLS * m + NTOK * b:COLS * m + NTOK * (b + 1)],
                                 pps[b][:, 0:NTOK], AF.Identity,
                                 bias=aux[:, 6 + m:7 + m])
    # V natural (+ ones cols): per (b, block) x 2 halves of 390
    for b in range(2):
        for t, (t0, ntk) in enumerate(VBLK):
            pps = [pp_p.tile([P, 512], F32, tag="pp", name=f"pp{i}") for i in range(2)]
            for k in range(6):
                x_sl = xn[:, COLS * k + NTOK * b + t0:COLS * k + NTOK * b + t0 + ntk]
                for nh in range(2):
                    nc.tensor.matmul(pps[nh][0:ntk, 0:390], x_sl,
                                     wvh[:, VW * k + 390 * nh:VW * k + 390 * (nh + 1)],
                                     start=(k == 0), stop=False)
            for nh in range(2):
                nc.tensor.matmul(pps[nh][0:ntk, 0:390], ones_r[0:1, 0:ntk],
                                 bv1[0:1, 390 * nh:390 * (nh + 1)],
                                 start=False, stop=True, skip_group_check=True)
                nc.scalar.activation(
                    vnat[0:ntk, VW * (4 * b + t) + 390 * nh:
                         VW * (4 * b + t) + 390 * (nh + 1)],
                    pps[nh][0:ntk, 0:390], AF.Copy)

    # wq region is dead after QKV; stream mlp_proj second half into it
    nc.sync.dma_start(wmh2, a["wmh2"])

    # ================= attention (k-major) =================
    for b in range(2):
        for h in range(NH):
            m, ro = h // 2, 64 * (h % 2)
            qts = qt[ro:ro + 64, 544 * m + LTOK * b:544 * m + LTOK * (b + 1)]
            stw = stw_p.tile([P, 512], F32, tag="stw")
            nc.tensor.matmul(stw[0:128, 0:LTOK],
                             kt[ro:ro + 64, COLS * m + NTOK * b + LTOK:
                                COLS * m + NTOK * (b + 1)],
                             qts, start=True, stop=True)
            std = std_p.tile([P, 512], F32, tag="std")
            for t, (g0, nq) in enumerate(GROUPS):
                nc.tensor.matmul(std[0:nq, g0:g0 + nq],
                                 kt[ro:ro + 64, COLS * m + NTOK * b + g0:
                                    COLS * m + NTOK * b + g0 + nq],
                                 qt[ro:ro + 64, 544 * m + LTOK * b + g0:
                                    544 * m + LTOK * b + g0 + nq],
                                 start=True, stop=True, skip_group_check=(t > 0))
            smw = smw_p.tile([P, LTOK], F32, tag="smw")
            nc.vector.tensor_tensor(smw[:, :], stw[0:128, 0:LTOK], maskw, OP.add)
            smd = smd_p.tile([P, LTOK], F32, tag="smd")
            nc.vector.tensor_tensor(smd[0:119, :], std[0:119, 0:LTOK],
                                    maskd[0:119, :], OP.add)
            ew = ew_p.tile([P, LTOK], BF16, tag="ew")
            nc.scalar.activation(ew[:, :], smw[:, :], AF.Exp)
            ed = ed_p.tile([P, LTOK], BF16, tag="ed")
            nc.scalar.activation(ed[0:119, :], smd[0:119, :], AF.Exp)
            av = av_p.tile([P, 512], F32, tag="av")
            nc.tensor.matmul(av[0:65, 0:LTOK],
                             vnat[0:128, VW * (4 * b + 3) + 65 * h:
                                  VW * (4 * b + 3) + 65 * (h + 1)],
                             ew[0:128, :], start=True, stop=False)
            for t, (g0, nq) in enumerate(GROUPS):
                nc.tensor.matmul(av[0:65, g0:g0 + nq],
                                 vnat[0:nq, VW * (4 * b + t) + 65 * h:
                                      VW * (4 * b + t) + 65 * (h + 1)],
                                 ed[0:nq, g0:g0 + nq],
                                 start=False, stop=(t == 2), skip_group_check=True)
            rr = rr_p.tile([1, LTOK], F32, tag="rr")
            nc.vector.reciprocal(rr[0:1, :], av[64:65, 0:LTOK])
            rbb = rbb_p.tile([64, LTOK], F32, tag="rbb")
            nc.gpsimd.partition_broadcast(rbb[:, :], rr[0:1, :])
            nc.vector.tensor_tensor(
                attnT[ro:ro + 64, 544 * m + LTOK * b:544 * m + LTOK * (b + 1)],
                av[0:64, 0:LTOK], rbb[:, :], OP.mult)

    if "dbg_xn" in a:
        nc.sync.dma_start(a["dbg_xn"], xn[:, :])
        nc.sync.dma_start(a["dbg_qt"], qt)
        nc.sync.dma_start(a["dbg_kt"], kt)
        nc.sync.dma_start(a["dbg_vnat"], vnat[:, :])
        nc.sync.dma_start(a["dbg_attnT"], attnT[:, :])

    # ================= attn_proj + residual =================
    for m in range(6):
        pps = [pp_p.tile([P, 512], F32, tag="pp", name=f"pp{i}") for i in range(2)]
        for k in range(6):
            w = wph[:, (m * 6 + k) * P:(m * 6 + k + 1) * P]
            for b in range(2):
                nc.tensor.matmul(pps[b][:, 0:LTOK], w,
                                 attnT[:, 544 * k + LTOK * b:544 * k + LTOK * (b + 1)],
                                 start=(k == 0), stop=(k == 5))
        for b in range(2):
            nc.vector.scalar_tensor_tensor(
                hid[:, 544 * m + LTOK * b:544 * m + LTOK * (b + 1)],
                pps[b][:, 0:LTOK], aux[:, 12 + m:13 + m],
                xsb[:, COLS * m + NTOK * b:COLS * m + NTOK * b + LTOK],
                op0=OP.add, op1=OP.add)

    # ================= LN2 =================
    ln_stats(hid, 544, LTOK)
    for k in range(6):
        tmp = tmp_p.tile([P, COLS], BF16, tag="tmp")
        nc.vector.tensor_tensor(tmp[:, 0:544], hid[:, 544 * k:544 * (k + 1)],
                                mub[:, 0:544], OP.subtract)
        nc.vector.tensor_tensor(h2n[:, 544 * k:544 * (k + 1)], tmp[:, 0:544],
                                rsb[:, 0:544], OP.mult)

    if "dbg_xn" in a:
        nc.sync.dma_start(a["dbg_hid"], hid[:, :])
        nc.sync.dma_start(a["dbg_h2n"], h2n)

    # ================= fc + gelu =================
    for m in range(24):
        pps = [pp_p.tile([P, 512], F32, tag="pp", name=f"pp{i}") for i in range(2)]
        for k in range(6):
            w = wfh[:, (m * 6 + k) * P:(m * 6 + k + 1) * P]
            for b in range(2):
                nc.tensor.matmul(pps[b][:, 0:LTOK], w,
                                 h2n[:, 544 * k + LTOK * b:544 * k + LTOK * (b + 1)],
                                 start=(k == 0), stop=(k == 5))
        for b in range(2):
            nc.scalar.activation(gelu[:, 544 * m + LTOK * b:544 * m + LTOK * (b + 1)],
                                 pps[b][:, 0:LTOK], AF.Gelu_apprx_tanh,
                                 bias=aux[:, 18 + m:19 + m])

    # ================= mlp_proj + residual + out =================
    for m in range(6):
        pps = [pp_p.tile([P, 512], F32, tag="pp", name=f"pp{i}") for i in range(2)]
        for k in range(24):
            if m < 3:
                w = wmh1[:, (m * 24 + k) * P:(m * 24 + k + 1) * P]
            else:
                w = wmh2[:, ((m - 3) * 24 + k) * P:((m - 3) * 24 + k + 1) * P]
            for b in range(2):
                nc.tensor.matmul(pps[b][:, 0:LTOK], w,
                                 gelu[:, 544 * k + LTOK * b:544 * k + LTOK * (b + 1)],
                                 start=(k == 0), stop=(k == 23))
        ys = ys_p.tile([P, 544], F32, tag="ys")
        for b in range(2):
            nc.vector.scalar_tensor_tensor(
                ys[:, LTOK * b:LTOK * (b + 1)],
                pps[b][:, 0:LTOK], aux[:, 42 + m:43 + m],
                hid[:, 544 * m + LTOK * b:544 * m + LTOK * (b + 1)],
                op0=OP.add, op1=OP.add)
        nc.sync.dma_start(a["yT"][P * m:P * (m + 1), :], ys[:, :])
    ctx.close()


def _build(dbg=False):
    nc = bacc.Bacc("TRN2", target_bir_lowering=False, debug=False)
    a = {}
    def din(name, shape, d=BF16):
        a[name] = nc.dram_tensor(name, shape, d, kind="ExternalInput").ap()
    if dbg:
        for nm, sh in [("dbg_xn", [128, 4800]), ("dbg_qt", [128, 3264]),
                       ("dbg_kt", [128, 4800]), ("dbg_vnat", [128, 8 * VW]),
                       ("dbg_attnT", [128, 3264]), ("dbg_hid", [128, 3264]),
                       ("dbg_h2n", [128, 3264])]:
            a[nm] = nc.dram_tensor(nm, sh, BF16, kind="ExternalOutput").ap()
    din("xT", [128, 4800])
    din("wqQ", [128, 4608])
    din("wqK", [128, 4608])
    din("wvh", [128, 6 * VW])
    din("wph", [128, 4608])
    din("wfh", [128, 18432])
    din("wmh1", [128, 9216])
    din("wmh2", [128, 9216])
    din("aux", [128, 48], F32)
    din("bv1", [1, VW])
    din("maskw", [128, LTOK])
    din("maskd", [128, LTOK])
    a["yT"] = nc.dram_tensor("yT", [HID, 544], F32, kind="ExternalOutput").ap()
    with tile.TileContext(nc) as tc:
        _body(tc, a)
    nc.compile()
    return nc


def _host_prep(inputs):
    x = np.asarray(inputs["hidden_states"], np.float32)
    ln1_g = np.asarray(inputs["ln1_g"], np.float32)
    ln1_b = np.asarray(inputs["ln1_b"], np.float32)
    ln2_g = np.asarray(inputs["ln2_g"], np.float32)
    ln2_b = np.asarray(inputs["ln2_b"], np.float32)
    caw = np.asarray(inputs["c_attn_w"], np.float32)
    apw = np.asarray(inputs["attn_proj_w"], np.float32)
    fcw = np.asarray(inputs["fc_w"], np.float32)
    mpw = np.asarray(inputs["mlp_proj_w"], np.float32)

    w1 = caw * ln1_g[:, None]                          # ln gain folded
    bqkv = ln1_b @ caw + np.asarray(inputs["c_attn_b"], np.float32)

    def chunk_mk(w, nk, nm):
        # [128k, 128m] -> [p, (m*nk+k)*128+f]
        return np.ascontiguousarray(
            w.reshape(nk, 128, nm, 128).transpose(1, 2, 0, 3).reshape(128, nk * nm * 128))

    wqQ = chunk_mk(w1[:, 0:768], 6, 6).astype(BF)
    wqK = chunk_mk(w1[:, 768:1536], 6, 6).astype(BF)
    wv_ext = np.zeros((768, VW), np.float32)
    for h in range(NH):
        wv_ext[:, 65 * h:65 * h + 64] = w1[:, 1536 + 64 * h:1536 + 64 * (h + 1)]
    wvh = np.ascontiguousarray(
        wv_ext.reshape(6, 128, VW).transpose(1, 0, 2).reshape(128, 6 * VW)).astype(BF)
    bv1 = np.zeros((1, VW), np.float32)
    bv1[0, 64::65] = 1.0
    wph = chunk_mk(apw, 6, 6).astype(BF)
    w2 = fcw * ln2_g[:, None]
    bfc = ln2_b @ fcw + np.asarray(inputs["fc_b"], np.float32)
    wfh = chunk_mk(w2, 6, 24).astype(BF)
    wmh = chunk_mk(mpw, 24, 6)
    wmh1 = np.ascontiguousarray(wmh[:, 0:9216]).astype(BF)
    wmh2 = np.ascontiguousarray(wmh[:, 9216:18432]).astype(BF)

    bv_vec = bqkv[1536:2304]
    bap2 = bv_vec @ apw + np.asarray(inputs["attn_proj_b"], np.float32)
    aux = np.zeros((128, 48), np.float32)
    aux[:, 0:6] = (0.125 * bqkv[0:768]).reshape(6, 128).T
    aux[:, 6:12] = bqkv[768:1536].reshape(6, 128).T
    aux[:, 12:18] = bap2.reshape(6, 128).T
    aux[:, 18:42] = bfc.reshape(24, 128).T
    aux[:, 42:48] = np.asarray(inputs["mlp_proj_b"], np.float32).reshape(6, 128).T

    # diag mask (core-independent): [r(k-in-group), g0+j] = 0 iff same
    # window and causal (j >= r)
    NEG = np.float32(-1e30)
    maskd_f = np.full((128, LTOK), NEG, np.float32)
    for (g0, nq) in GROUPS:
        for r in range(nq):
            lo = (r // WIN) * WIN
            hi = min(lo + WIN, nq)
            maskd_f[r, g0 + r:g0 + hi] = 0.0
            assert max(r, lo) == r
    maskd = maskd_f.astype(BF)

    wh_idx = np.arange(WH) * WIN
    shared = dict(wqQ=wqQ, wqK=wqK, wvh=wvh, wph=wph, wfh=wfh,
                  wmh1=wmh1, wmh2=wmh2, aux=aux, bv1=bv1.astype(BF),
                  maskd=maskd)
    in_maps = []
    for c in range(NCORE):
        t0 = LTOK * c
        xcat = np.concatenate([x[0, t0:t0 + LTOK], x[0, wh_idx],
                               x[1, t0:t0 + LTOK], x[1, wh_idx]], 0)  # [800, 768]
        xT = np.ascontiguousarray(
            xcat.T.reshape(6, 128, COLS).transpose(1, 0, 2).reshape(128, 4800)).astype(BF)
        blk_q = 16 * c + (np.arange(LTOK) // WIN)
        maskw = np.where(np.arange(WH)[:, None] < blk_q[None, :], 0.0, NEG)
        in_maps.append(dict(shared, xT=xT, maskw=maskw.astype(BF)))
    return in_maps


def kernel(**inputs):
    global LAST_RESULTS
    if "nc" not in _CACHE:
        _CACHE["nc"] = _build()
    nc = _CACHE["nc"]
    in_maps = _host_prep(inputs)
    res = run_bass_kernel_spmd(nc, in_maps, core_ids=list(range(NCORE)))
    LAST_RESULTS = res
    out = np.zeros((B, S, HID), np.float32)
    for c in range(NCORE):
        yT = res.results[c]["yT"]           # [768, 544]
        t0 = LTOK * c
        out[0, t0:t0 + LTOK] = yT[:, :LTOK].T
        out[1, t0:t0 + LTOK] = yT[:, LTOK:].T
    return out
